# revision 2
# baseline (speedup 1.0000x reference)
"""TRN2 Bass kernel for the 4-layer encoder-with-reaches model
(nn_EncoderPreTre, B=8, S=512, D=1024, H=16, NL=4).

Contract: kernel(**inputs) takes the FULL inputs (src [8,512] int32,
reaches [8,512] f32, emb_table [32000,1024] f32, qw/kw/vw/ow [4,1024,1024]
f32) and returns the full output tuple (emb [8,512,1024] f32,
x [8,512,1024] f32), matching reference.reference().

Distribution: data-parallel over the batch — core b computes batch
element b end to end (B == 8 == n_cores). The embedding-row gather and
per-batch contrib/reaches vectors are the host-side sharding step; all
transformer compute (4 layers of projections + reaches-attention +
output projection) runs on the NeuronCores in fp32. fp32 is required:
this model's logits reach 5.6e6 and its contrib scaling grows x ~40x per
layer, so softmax amplifies matmul rounding ~1000x — bf16/tf32 (and
TRN2's float32r) matmuls fail a 2e-2 gate; fp32 lands ~1e-4.

Per-core dataflow (residual kept transposed: xT [1024,512] fp32 SBUF):
  P1: qT = (qw/8)^T-proj, kT likewise, v in [s,do] layout with
      v' = v*reaches fused into the PSUM->SBUF copy.
  P2 per head: scores[q,k] (one matmul per q-chunk, K=64) ->
      row max via DVE reduce (negated) -> E = exp(s - m) with the row sum
      Z from the same ACT op (accum_out) — numerator and denominator use
      the same PSUM values, keeping the softmax consistent at huge logit
      scale -> P = (E * (-c/Z)[q]) * diagmask in one scalar_tensor_tensor
      -> P transposed 128x128-blockwise via PE transpose-mode ->
      M2T[dk,q] = sum_k v'[k,dk]*PT[k,q] accumulated over 4 k-chunks.
  P3: x += (OV@x)*c + ow-proj(concatT), with OV = ow@vw folded on the
      host so the reference's "v - P@v'" becomes two accumulating
      projections (concatT carries -c*(P_norm@v')^T).
"""
import numpy as np

import concourse.tile as tile
from concourse import bacc, mybir
from concourse.bass_utils import run_bass_kernel_spmd

F32 = mybir.dt.float32
AX = mybir.AxisListType
OP = mybir.AluOpType
AF = mybir.ActivationFunctionType

B, S, D, H, DK, NL = 8, 512, 1024, 16, 64, 4
QC = S // 128
KC = S // 128
DC = D // 128

TRACE = False        # test harness sets True to capture a neuron profile
LAST_RESULT = None   # BassKernelResults of the last kernel() call
_NC_CACHE = {}


def _build(n_cores=B):
    nc = bacc.Bacc("TRN2", target_bir_lowering=False, debug=False,
                   num_devices=n_cores)
    d_x0 = nc.dram_tensor("x0t", [D, S], F32, kind="ExternalInput").ap()
    d_wq = nc.dram_tensor("wq", [NL, D, D], F32, kind="ExternalInput").ap()
    d_wk = nc.dram_tensor("wk", [NL, D, D], F32, kind="ExternalInput").ap()
    d_wv = nc.dram_tensor("wv", [NL, D, D], F32, kind="ExternalInput").ap()
    d_wo = nc.dram_tensor("wo", [NL, D, D], F32, kind="ExternalInput").ap()
    d_wov = nc.dram_tensor("wov", [NL, D, D], F32, kind="ExternalInput").ap()
    d_cb = nc.dram_tensor("cb", [128, S], F32, kind="ExternalInput").ap()
    d_negc = nc.dram_tensor("negc", [128, QC], F32, kind="ExternalInput").ap()
    d_rr = nc.dram_tensor("rr", [128, KC], F32, kind="ExternalInput").ap()
    d_mask = nc.dram_tensor("maskq", [QC, 128, S], F32, kind="ExternalInput").ap()
    d_id = nc.dram_tensor("ident", [128, 128], F32, kind="ExternalInput").ap()
    d_out = nc.dram_tensor("xt", [D, S], F32, kind="ExternalOutput").ap()

    with tile.TileContext(nc) as tc:
        _emit(nc, tc, d_x0, d_wq, d_wk, d_wv, d_wo, d_wov,
              d_cb, d_negc, d_rr, d_mask, d_id, d_out)
    nc.compile()
    return nc


def _emit(nc, tc, d_x0, d_wq, d_wk, d_wv, d_wo, d_wov,
          d_cb, d_negc, d_rr, d_mask, d_id, d_out):
    ctx_pools = []

    def pool(name, bufs, space="SBUF"):
        p = tc.tile_pool(name=name, bufs=bufs, space=space)
        ctx_pools.append(p)
        return p.__enter__()

    const = pool("const", 1)
    xpool = pool("x", 1)
    actp = pool("act", 1)         # qt/kt/vp/concatT, persistent per layer
    wpool = pool("w", 2)          # weight double buffer per ki-tile
    epool = pool("E", 2)
    ppool = pool("P", 4)
    ptpool = pool("PT", 5)
    small = pool("small", 3)
    tmp8 = pool("tmp8", 2)
    psA = pool("psA", 3, "PSUM")  # projections + scores
    psB = pool("psB", 2, "PSUM")  # transposes + ow-proj
    psC = pool("psC", 2, "PSUM")  # M2 + OV-proj

    cb = const.tile([128, S], F32)
    nc.sync.dma_start(cb[:], d_cb)
    negc = const.tile([128, QC], F32)
    nc.sync.dma_start(negc[:], d_negc)
    rr = const.tile([128, KC], F32)
    nc.sync.dma_start(rr[:], d_rr)
    ident = const.tile([128, 128], F32)
    nc.sync.dma_start(ident[:], d_id)
    masks = []
    for t in range(QC):
        mt = const.tile([128, S], F32, tag=f"mask{t}", name=f"mask{t}")
        nc.sync.dma_start(mt[:], d_mask[t])
        masks.append(mt)

    xT = []
    for c in range(DC):
        xc = xpool.tile([128, S], F32, tag=f"x{c}", name=f"x{c}")
        nc.sync.dma_start(xc[:], d_x0[c * 128:(c + 1) * 128, :])
        xT.append(xc)

    for l in range(NL):
        def load_w(dram):
            tiles = []
            for ki in range(DC):
                wt = wpool.tile([128, D], F32, tag=f"w{ki}", name=f"w{ki}_{l}")
                nc.sync.dma_start(wt[:], dram[l, ki * 128:(ki + 1) * 128, :])
                tiles.append(wt)
            return tiles

        def proj_T(wtiles, outtag):
            # out[do, s]: lhsT = w[di-chunk, do-slice], rhs = xT[di-chunk]
            outs = []
            for c in range(DC):
                p = psA.tile([128, S], F32, tag="psA", name=f"pp{outtag}{c}_{l}")
                for ki in range(DC):
                    nc.tensor.matmul(
                        p[:], wtiles[ki][:, c * 128:(c + 1) * 128], xT[ki][:],
                        start=(ki == 0), stop=(ki == DC - 1))
                o = actp.tile([128, S], F32, tag=f"{outtag}{c}",
                              name=f"{outtag}{c}_{l}")
                nc.vector.tensor_copy(o[:], p[:])
                outs.append(o)
            return outs

        qt = proj_T(load_w(d_wq), "qt")
        kt = proj_T(load_w(d_wk), "kt")

        wv_t = load_w(d_wv)
        vp = []
        for sc in range(KC):
            vtile = actp.tile([128, D], F32, tag=f"vp{sc}", name=f"vp{sc}_{l}")
            for half in range(2):
                p = psA.tile([128, S], F32, tag="psA", name=f"pv{sc}{half}_{l}")
                for ki in range(DC):
                    nc.tensor.matmul(
                        p[:], xT[ki][:, sc * 128:(sc + 1) * 128],
                        wv_t[ki][:, half * 512:(half + 1) * 512],
                        start=(ki == 0), stop=(ki == DC - 1))
                nc.vector.tensor_scalar(
                    vtile[:, half * 512:(half + 1) * 512], p[:],
                    rr[:, sc:sc + 1], None, op0=OP.mult)
            vp.append(vtile)

        concatT = [actp.tile([128, S], F32, tag=f"cc{c}", name=f"cc{c}_{l}")
                   for c in range(DC)]
        for h in range(H):
            hp = h // 2
            hb = (h % 2) * 64
            qsl = qt[hp][hb:hb + 64, :]
            ksl = kt[hp][hb:hb + 64, :]

            negm = small.tile([128, QC], F32, tag="negm", name=f"negm{h}_{l}")
            zst = small.tile([128, QC], F32, tag="zst", name=f"zst{h}_{l}")
            sc_t = small.tile([128, QC], F32, tag="scl", name=f"scl{h}_{l}")
            Ps = []
            for t in range(QC):
                ps = psA.tile([128, S], F32, tag="psA", name=f"sc{h}{t}_{l}")
                nc.tensor.matmul(ps[:], qsl[:, t * 128:(t + 1) * 128], ksl,
                                 start=True, stop=True)
                nc.vector.tensor_reduce(
                    negm[:, t:t + 1], ps[:], axis=AX.X, op=OP.max, negate=True)
                e = epool.tile([128, S], F32, tag="E", name=f"e{h}{t}_{l}")
                nc.scalar.activation(e[:], ps[:], AF.Exp,
                                     bias=negm[:, t:t + 1], scale=1.0,
                                     accum_out=zst[:, t:t + 1])
                nc.vector.reciprocal(sc_t[:, t:t + 1], zst[:, t:t + 1])
                nc.vector.tensor_tensor(
                    sc_t[:, t:t + 1], sc_t[:, t:t + 1], negc[:, t:t + 1],
                    op=OP.mult)
                p = ppool.tile([128, S], F32, tag="P", name=f"p{h}{t}_{l}")
                nc.vector.scalar_tensor_tensor(
                    p[:], e[:], sc_t[:, t:t + 1], masks[t][:],
                    op0=OP.mult, op1=OP.mult)
                Ps.append(p)

            PTs = []
            for kc in range(KC):
                tp = psB.tile([128, S], F32, tag="psB", name=f"tp{h}{kc}_{l}")
                for t in range(QC):
                    nc.tensor.matmul(
                        tp[:, t * 128:(t + 1) * 128],
                        Ps[t][:, kc * 128:(kc + 1) * 128], ident[:],
                        is_transpose=True, start=(t == 0), stop=(t == QC - 1),
                        skip_group_check=True)
                pt_sb = ptpool.tile([128, S], F32, tag="PT",
                                    name=f"pt{h}{kc}_{l}")
                if kc % 2 == 0:
                    nc.vector.tensor_copy(pt_sb[:], tp[:])
                else:
                    nc.scalar.copy(pt_sb[:], tp[:])
                PTs.append(pt_sb)

            m2 = psC.tile([128, S], F32, tag="psC", name=f"m2{h}_{l}")
            off = hb
            for kc in range(KC):
                nc.tensor.matmul(
                    m2[off:off + 64, :], vp[kc][:, h * 64:h * 64 + 64],
                    PTs[kc][:], start=(kc == 0), stop=(kc == KC - 1))
            nc.vector.tensor_copy(concatT[hp][hb:hb + 64, :], m2[off:off + 64, :])

        wov_t = load_w(d_wov)
        wo_t = load_w(d_wo)
        # OV pass first: stage (OV@x)*c in SBUF so the later in-place xT
        # updates cannot race the OV matmuls (they read pre-update xT).
        t1s = []
        for c in range(DC):
            pov = psC.tile([128, S], F32, tag="psC", name=f"pov{c}_{l}")
            for ki in range(DC):
                nc.tensor.matmul(
                    pov[:], wov_t[ki][:, c * 128:(c + 1) * 128], xT[ki][:],
                    start=(ki == 0), stop=(ki == DC - 1))
            t1 = tmp8.tile([128, S], F32, tag=f"t1{c}", name=f"t1{c}_{l}",
                           bufs=1)
            nc.vector.tensor_tensor(t1[:], pov[:], cb[:], op=OP.mult)
            t1s.append(t1)
        for c in range(DC):
            pow_ = psB.tile([128, S], F32, tag="psB", name=f"pow{c}_{l}")
            for ki in range(DC):
                nc.tensor.matmul(
                    pow_[:], wo_t[ki][:, c * 128:(c + 1) * 128], concatT[ki][:],
                    start=(ki == 0), stop=(ki == DC - 1))
            nc.vector.tensor_tensor(xT[c][:], xT[c][:], pow_[:], op=OP.add)
            nc.vector.tensor_tensor(xT[c][:], xT[c][:], t1s[c][:], op=OP.add)

    for c in range(DC):
        nc.sync.dma_start(d_out[c * 128:(c + 1) * 128, :], xT[c][:])

    for p in reversed(ctx_pools):
        p.__exit__(None, None, None)


def _host_prep(src, reaches, emb_table, qw, kw, vw, ow):
    src = np.asarray(src)
    reaches = np.asarray(reaches, dtype=np.float32)
    emb_table = np.asarray(emb_table, dtype=np.float32)
    emb = emb_table[src]                       # [B,S,D] == output 0
    rs = reaches.sum(-1, keepdims=True)
    contrib = ((rs - reaches) / (rs + 1e-9) * (1.0 - reaches) * 100.0
               ).astype(np.float32)

    qw = np.asarray(qw, np.float32); kw = np.asarray(kw, np.float32)
    vw = np.asarray(vw, np.float32); ow = np.asarray(ow, np.float32)
    wq = np.ascontiguousarray(np.transpose(qw, (0, 2, 1)) * 0.125)
    wk = np.ascontiguousarray(np.transpose(kw, (0, 2, 1)))
    wv = np.ascontiguousarray(np.transpose(vw, (0, 2, 1)))
    wo = np.ascontiguousarray(np.transpose(ow, (0, 2, 1)))
    wov = np.stack([
        np.ascontiguousarray(
            (ow[l].astype(np.float64) @ vw[l].astype(np.float64)).T
        ).astype(np.float32)
        for l in range(NL)])

    maskq = np.ones((QC, 128, S), np.float32)
    idx = np.arange(128)
    diagval = np.float32(1.0) - np.float32(0.999999)  # as the fp32 ref computes
    for t in range(QC):
        maskq[t, idx, t * 128 + idx] = diagval
    ident = np.eye(128, dtype=np.float32)

    shared = dict(wq=wq, wk=wk, wv=wv, wo=wo, wov=wov,
                  maskq=maskq, ident=ident)
    in_maps = []
    for b in range(B):
        in_maps.append(dict(
            shared,
            x0t=np.ascontiguousarray(emb[b].T),
            cb=np.ascontiguousarray(
                np.broadcast_to(contrib[b][None, :], (128, S))),
            negc=np.ascontiguousarray(-contrib[b].reshape(QC, 128).T),
            rr=np.ascontiguousarray(reaches[b].reshape(KC, 128).T),
        ))
    return emb, in_maps


def kernel(src, reaches, emb_table, qw, kw, vw, ow):
    global LAST_RESULT
    if "nc" not in _NC_CACHE:
        _NC_CACHE["nc"] = _build(n_cores=B)
    nc = _NC_CACHE["nc"]
    emb, in_maps = _host_prep(src, reaches, emb_table, qw, kw, vw, ow)
    res = run_bass_kernel_spmd(nc, in_maps, core_ids=list(range(B)),
                               trace=TRACE)
    LAST_RESULT = res
    x = np.stack([r["xt"].T for r in res.results]).astype(np.float32)
    return emb, x
